# revision 67
# baseline (speedup 1.0000x reference)
"""GLM-style dual-RoPE attention block on 8 trn2 NeuronCores.

Sharding: tensor-parallel over heads (16 heads -> 2 per core).
Per core: QKV projection for its heads (transposed layout), dual RoPE,
full S x S attention (streamed softmax over key tiles, no max subtraction
-- max |logit| ~60 so exp stays in fp32 range), unnormalized P@V,
late normalization, and a partial output projection.  Partials are summed
on host; qkv v-bias is folded into a host-side constant row, attn_out
bias added on host.

All matmuls run in bf16 (measured 227 ns per [128x128]@[128x512] vs
427 ns for f32r at ramped clock -- f32r is SBUF-bandwidth-bound at peak
frequency).  RoPE is computed as qk = (psum+b)*cos + swap32((psum+b)*sinP)
where sinP has the rotate-half sign folded in on host and swap32 is a
4x[32,512] SBUF->SBUF DMA partition swap -- no ACT-engine rotate copies.
The softmax denominator comes from a bf16 running accumulation of the
exp tiles on DVE plus two [1,512] ones-matmuls per (head, 1024-query
block), replacing the per-key-tile ones-matmuls (saves ~60k PE rows).
The output projection for each 1024-query block is emitted right after
that block's attention so the PE never drains until the very end.
"""

import ml_dtypes
import numpy as np

import concourse.bass as bass
from concourse import bacc
import concourse.mybir as mybir
import concourse.tile as tile
from concourse.bass_utils import run_bass_kernel_spmd
from concourse.masks import make_identity

F32 = mybir.dt.float32
BF16 = mybir.dt.bfloat16
F16 = mybir.dt.float16
AF = mybir.ActivationFunctionType
OP = mybir.AluOpType

S, D, H, HD = 2048, 2048, 16, 128
NCORES = 8
HPC = H // NCORES          # heads per core = 2
KT = D // 128              # 16 contraction tiles
ST = S // 128              # 16 sequence 128-tiles
QC = S // 512              # 4 sequence 512-chunks

_LAST_RESULTS = None
_BUILT = None


def _build():
    nc = bacc.Bacc("TRN2", target_bir_lowering=False, debug=False,
                   num_devices=NCORES)
    xT_d = nc.dram_tensor("xT", [D, S], F16, kind="ExternalInput").ap()
    wqk_d = nc.dram_tensor("wqk", [D, 4 * 128], F16, kind="ExternalInput").ap()
    bqk_d = nc.dram_tensor("bqk", [128, 4], F32, kind="ExternalInput").ap()
    wv_d = nc.dram_tensor("wv", [D, HPC * 128], F16, kind="ExternalInput").ap()
    cos_d = nc.dram_tensor("cos", [128, S], F16, kind="ExternalInput").ap()
    sin_d = nc.dram_tensor("sin", [128, S], F16, kind="ExternalInput").ap()
    wo_d = nc.dram_tensor("wo", [HPC * 128, D], BF16, kind="ExternalInput").ap()
    out_d = nc.dram_tensor("out", [S, D], BF16, kind="ExternalOutput").ap()

    with tile.TileContext(nc) as tc:
        with tc.tile_pool(name="res", bufs=1) as res:
            cos_sb = res.tile([128, S], F16, tag="cos")
            sin_sb = res.tile([128, S], F16, tag="sin")
            bqk_sb = res.tile([128, 4], F32, tag="bqk")
            wo_sb = [res.tile([128, D], BF16, tag=f"wo{h}", name=f"wo{h}")
                     for h in range(HPC)]
            qkT = [[res.tile([128, 512], F16, tag=f"qkT{m}_{nq}",
                             name=f"qkT{m}_{nq}") for nq in range(QC)]
                   for m in range(4)]
            vnat = [res.tile([128, 256], BF16, tag=f"vnat{st}",
                             name=f"vnat{st}") for st in range(ST)]
            wvn = [[res.tile([128, 512], BF16, tag=f"wvn{h}_{nq}",
                             name=f"wvn{h}_{nq}") for nq in range(QC)]
                   for h in range(HPC)]

            # ---- phase 1: qkv^T = W^T @ x^T (streamed over s-quarters) ----
            with (
                tc.tile_pool(name="xs", bufs=4) as xs,
                tc.tile_pool(name="tmp", bufs=3) as tmp,
                tc.tile_pool(name="vt", bufs=1) as vtp,
                tc.tile_pool(name="ps1", bufs=8, space="PSUM") as ps1,
            ):
                wqk_sb = [xs.tile([128, 512], F16, tag=f"wqk{k}",
                                  name=f"wqk{k}", bufs=1) for k in range(KT)]
                wv_sb = [xs.tile([128, 256], F16, tag=f"wvw{k}",
                                 name=f"wvw{k}", bufs=1) for k in range(KT)]
                vT = [[vtp.tile([128, 512], BF16, tag=f"vT{h}_{nq}",
                                name=f"vT{h}_{nq}") for nq in range(QC)]
                      for h in range(HPC)]
                ones_f = res.tile([128, 128], F32, tag="ones_f")
                nc.gpsimd.memset(ones_f[:], 1.0)
                ones_sb = res.tile([128, 128], BF16, tag="ones")
                nc.vector.tensor_copy(ones_sb[:], ones_f[:])
                ident_f = res.tile([128, 128], F32, tag="ident_f")
                make_identity(nc, ident_f[:])
                ident = res.tile([128, 128], BF16, tag="ident")
                nc.vector.tensor_copy(ident[:], ident_f[:])

                def rope_evict(m, nq, psums):
                    # single cheap op frees the PSUM bank in ~0.7us; the rest
                    # of the rope then runs from SBUF at 2x fp16 DVE rate
                    zb = tmp.tile([128, 512], F16, tag="zb", bufs=5,
                                  name=f"zb{m}_{nq}")
                    nc.vector.tensor_scalar_add(zb[:], psums[m][:],
                                                bqk_sb[:, m:m + 1])
                    return zb

                def rope_finish(m, nq, zb):
                    ns = slice(nq * 512, (nq + 1) * 512)
                    ts = tmp.tile([128, 512], F16, tag="ts", name=f"ts{m}_{nq}")
                    nc.vector.tensor_mul(ts[:], zb[:], sin_sb[:, ns])
                    tp = tmp.tile([128, 512], F16, tag="tp", name=f"tp{m}_{nq}")
                    for blk in range(2):
                        b0 = blk * 64
                        nc.gpsimd.dma_start(tp[b0:b0 + 32, :],
                                            ts[b0 + 32:b0 + 64, :])
                        nc.gpsimd.dma_start(tp[b0 + 32:b0 + 64, :],
                                            ts[b0:b0 + 32, :])
                    t1 = tmp.tile([128, 512], F16, tag="t1", name=f"t1{m}_{nq}")
                    nc.vector.tensor_mul(t1[:], zb[:], cos_sb[:, ns])
                    nc.vector.tensor_add(qkT[m][nq][:], t1[:], tp[:])

                for nq in range(QC):
                    ns = slice(nq * 512, (nq + 1) * 512)
                    psums = [ps1.tile([128, 512], F32, tag="ps",
                                      name=f"qkvps{nq}_{i}", bufs=8)
                             for i in range(6)]
                    for k in range(KT):
                        if nq == 0:
                            # weights on the scalar DMA queue so nq=0 is not
                            # paced by a single queue
                            nc.scalar.dma_start(wqk_sb[k][:],
                                                wqk_d[k * 128:(k + 1) * 128, :])
                            nc.scalar.dma_start(wv_sb[k][:],
                                                wv_d[k * 128:(k + 1) * 128, :])
                        xt = xs.tile([128, 512], F16, tag="xt", bufs=8)
                        # first tiles split across two queues so the PE isn't
                        # DMA-starved during the k=0..3 warmup
                        xq = nc.gpsimd if (nq == 0 and k in (1, 3)) else nc.sync
                        xq.dma_start(xt[:], xT_d[k * 128:(k + 1) * 128, ns])
                        if nq == 0 and k == 8:
                            # tables + wo on the gpsimd DMA queue, mid-way so
                            # they don't compete with the first xt tiles
                            nc.gpsimd.dma_start(cos_sb[:], cos_d[:, :])
                            nc.gpsimd.dma_start(sin_sb[:], sin_d[:, :])
                            nc.gpsimd.dma_start(bqk_sb[:], bqk_d[:, :])
                            for h in range(HPC):
                                nc.gpsimd.dma_start(
                                    wo_sb[h][:],
                                    wo_d[h * 128:(h + 1) * 128, :])
                        # v first in the last k-group: its psums close
                        # earliest so vT copies + transposes start sooner
                        order = ((4, 5, 2, 3, 0, 1) if k == KT - 1
                                 else (2, 3, 0, 1, 4, 5))
                        for m in order:
                            w = (wqk_sb[k][:, m * 128:(m + 1) * 128] if m < 4
                                 else wv_sb[k][:, (m - 4) * 128:
                                              (m - 3) * 128])
                            nc.tensor.matmul(
                                psums[m][:], w, xt[:],
                                start=(k == 0), stop=(k == KT - 1))
                    # evict all qk psums first (frees banks fastest), v next
                    zbs = {m: rope_evict(m, nq, psums) for m in (2, 3, 0, 1)}
                    for h in range(HPC):
                        nc.scalar.copy(vT[h][nq][:], psums[4 + h][:])
                    for h in range(HPC):
                        for j in range(4):
                            st = nq * 4 + j
                            tp2 = ps1.tile([128, 128], BF16, tag="ps", bufs=8,
                                           name=f"tp2_{h}_{st}")
                            nc.tensor.transpose(
                                tp2[:],
                                vT[h][nq][:, j * 128:(j + 1) * 128],
                                ident[:])
                            nc.scalar.copy(vnat[st][:, h * 128:(h + 1) * 128],
                                           tp2[:])
                    for m in (2, 3, 0, 1):
                        rope_finish(m, nq, zbs[m])

            # ---- phase 2 + 3: attention and out-projection, per 1024-q ----
            with (
                tc.tile_pool(name="ex", bufs=4) as exp_pool,
                tc.tile_pool(name="ac", bufs=2) as acp,
                tc.tile_pool(name="rp", bufs=2) as rp,
                tc.tile_pool(name="ob", bufs=2) as obp,
                tc.tile_pool(name="ps2", bufs=1, space="PSUM") as ps2,
                tc.tile_pool(name="ps3", bufs=2, space="PSUM") as ps3,
            ):
                # out-projection slots for completed 1024-q blocks; popped
                # one per key-tile inside later attention blocks so the PE
                # fills the exp-cadence stalls instead of idling
                ph3 = []
                obs = {}

                def emit_ph3():
                    if not ph3:
                        return
                    qt, oc = ph3.pop(0)
                    if oc == 0:
                        obs[qt] = obp.tile([128, D], BF16, tag="ob",
                                           name=f"ob{qt}")
                    ob = obs[qt]
                    op = ps3.tile([128, 512], F32, tag="op", bufs=2,
                                  name=f"op{qt}_{oc}")
                    for h2 in range(HPC):
                        nc.tensor.matmul(
                            op[:],
                            wvn[h2][qt // 4][:, (qt % 4) * 128:
                                             (qt % 4 + 1) * 128],
                            wo_sb[h2][:, oc * 512:(oc + 1) * 512],
                            start=(h2 == 0), stop=(h2 == HPC - 1))
                    if oc % 2 == 0:
                        nc.vector.tensor_copy(ob[:, oc * 512:(oc + 1) * 512],
                                              op[:])
                    else:
                        nc.scalar.copy(ob[:, oc * 512:(oc + 1) * 512], op[:])
                    if oc == 1:
                        nc.sync.dma_start(out_d[qt * 128:(qt + 1) * 128,
                                                0:1024], ob[:, 0:1024])
                    elif oc == 3:
                        nc.sync.dma_start(out_d[qt * 128:(qt + 1) * 128,
                                                1024:2048], ob[:, 1024:2048])
                        obs.pop(qt)

                for qc in range(2):
                    for h in range(HPC):
                        qT_h = qkT[h]
                        kT_h = qkT[2 + h]
                        wv_ps = ps2.tile([128, 1024], F32, tag="wv", bufs=1,
                                         name=f"wvps{h}_{qc}")
                        acc = acp.tile([128, 1024], BF16, tag="acc")
                        exs = {}
                        for st in range(ST + 2):   # 2-deep software pipeline
                            if st < ST:
                                lg = ps2.tile([128, 1024], F32, tag="lg",
                                              bufs=2, name=f"lg{h}_{qc}_{st}")
                                kts = kT_h[st // 4][:, (st % 4) * 128:
                                                    (st % 4 + 1) * 128]
                                for half in range(2):
                                    nc.tensor.matmul(
                                        lg[:, half * 512:(half + 1) * 512],
                                        kts,
                                        qT_h[2 * qc + half][:],
                                        start=True, stop=True)
                                ex = exp_pool.tile([128, 1024], BF16,
                                                   tag="ex")
                                nc.scalar.activation(ex[:], lg[:], AF.Exp)
                                if st == 0:
                                    nc.vector.tensor_copy(acc[:], ex[:])
                                else:
                                    nc.vector.tensor_add(acc[:], acc[:], ex[:])
                                exs[st] = ex
                            if st >= 2:
                                ex = exs.pop(st - 2)
                                sp = st - 2
                                for half in range(2):
                                    exh = ex[:, half * 512:(half + 1) * 512]
                                    nc.tensor.matmul(
                                        wv_ps[:, half * 512:(half + 1) * 512],
                                        vnat[sp][:, h * 128:(h + 1) * 128],
                                        exh,
                                        start=(sp == 0), stop=(sp == ST - 1))
                                if st >= 4:
                                    emit_ph3()
                            if st == ST + 1:
                                # denominator after PV(15) in queue order; the
                                # all-ones stationary reduces AND broadcasts
                                # across partitions in the matmul itself
                                sm = ps2.tile([128, 1024], F32, tag="lg",
                                              bufs=2, name=f"sm{h}_{qc}")
                                for half in range(2):
                                    nc.tensor.matmul(
                                        sm[:, half * 512:(half + 1) * 512],
                                        ones_sb[:],
                                        acc[:, half * 512:(half + 1) * 512],
                                        start=True, stop=True)
                                rc = rp.tile([128, 1024], F32, tag="rc",
                                             bufs=2, name=f"rc{h}_{qc}")
                                nc.vector.reciprocal_approx_fast(rc[:], sm[:])
                        for half in range(2):
                            nc.vector.tensor_mul(
                                wvn[h][2 * qc + half][:],
                                wv_ps[:, half * 512:(half + 1) * 512],
                                rc[:, half * 512:(half + 1) * 512])
                    ph3.extend((qt, oc)
                               for qt in range(8 * qc, 8 * qc + 8)
                               for oc in range(4))
                while ph3:
                    emit_ph3()

    nc.compile()
    return nc


def kernel(x, qkv_weight, qkv_bias, attn_out_weight, attn_out_bias,
           position_ids):
    global _BUILT, _LAST_RESULTS
    x = np.asarray(x, np.float32)
    qkv_weight = np.asarray(qkv_weight, np.float32)
    qkv_bias = np.asarray(qkv_bias, np.float32)
    attn_out_weight = np.asarray(attn_out_weight, np.float32)
    attn_out_bias = np.asarray(attn_out_bias, np.float32)
    position_ids = np.asarray(position_ids)

    half = HD // 2
    xT = np.ascontiguousarray(x[:, 0, :].T.astype(np.float16))
    inv_freq = 1.0 / (10000.0 ** (np.arange(0, half, 2, dtype=np.float32) / half))
    pos1 = position_ids[0, 0, :].astype(np.float32)
    pos2 = position_ids[0, 1, :].astype(np.float32)
    ang1 = np.concatenate([inv_freq[:, None] * pos1[None, :]] * 2, axis=0)
    ang2 = np.concatenate([inv_freq[:, None] * pos2[None, :]] * 2, axis=0)
    COS = np.concatenate([np.cos(ang1), np.cos(ang2)], axis=0)
    SIN = np.concatenate([np.sin(ang1), np.sin(ang2)], axis=0)
    # fold the rotate-half sign into sin: rows 32:64 and 96:128 negated
    sign = np.ones((128, 1), np.float32)
    sign[32:64] = -1.0
    sign[96:128] = -1.0
    SINP = np.ascontiguousarray((SIN * sign).astype(np.float16))
    COS = np.ascontiguousarray(COS.astype(np.float16))

    in_maps = []
    for c in range(NCORES):
        c0 = c * HPC * HD                     # first q column of this core
        wq = qkv_weight[:, c0:c0 + HPC * HD]
        wk = qkv_weight[:, D + c0:D + c0 + HPC * HD]
        wv = qkv_weight[:, 2 * D + c0:2 * D + c0 + HPC * HD]
        bq = qkv_bias[c0:c0 + HPC * HD]
        bk = qkv_bias[D + c0:D + c0 + HPC * HD]
        wo = attn_out_weight[c0:c0 + HPC * HD, :]
        wqk = np.ascontiguousarray(
            np.concatenate([wq, wk], axis=1).astype(np.float16))
        bqk = np.ascontiguousarray(
            np.stack([bq[:128], bq[128:], bk[:128], bk[128:]], axis=1))
        in_maps.append({
            "xT": xT,
            "wqk": wqk,
            "bqk": bqk,
            "wv": np.ascontiguousarray(wv.astype(np.float16)),
            "cos": COS,
            "sin": SINP,
            "wo": np.ascontiguousarray(wo.astype(ml_dtypes.bfloat16)),
        })

    if _BUILT is None:
        _BUILT = _build()
    res = run_bass_kernel_spmd(_BUILT, in_maps, core_ids=list(range(NCORES)))
    _LAST_RESULTS = res

    acc = np.zeros((S, D), dtype=np.float32)
    for r in res.results:
        acc += r["out"].astype(np.float32)
    bv = qkv_bias[2 * D:3 * D]
    acc += (bv @ attn_out_weight)[None, :] + attn_out_bias[None, :]
    return acc.reshape(S, 1, D).astype(np.float32)
